# revision 29
# baseline (speedup 1.0000x reference)
"""Trainium2 Bass kernel for the per-task embedding MLP (embedding_lookup).

Computation (per sample j with task t = task_ids[j]):
    h      = x[j] @ l1_emb[t].reshape(256, 128) + l1_bias[t]
    g      = gelu_exact(h)
    out[j] = sum(g * l2_emb[t]) + l2_bias[t, 0]

Strategy: shard the *task* axis across the 8 cores (125 contiguous tasks per
core), so each core streams a contiguous slab of l1_emb exactly once (the
memory roofline), instead of gathering a 128 KiB row per sample (4x more
traffic).  Samples are routed (host-side index math only) to the core owning
their task and packed into a fixed slot grid of W columns per task, so all 8
cores run one identical SPMD program: per task, two K=128 matmuls of the
task's [256,128] weights against its [256,W] x-columns accumulate
hT[128, 125*W] in PSUM; the epilogue does bias-add + erf-gelu + w2-mult on
column-broadcast views and reduces over hidden via a ones-vector matmul.

The stage-1 matmul operands (x, w1) are cast to fp16 on the host: fp32
matmuls on trn2 lower to LOW/HIGH double passes (~460 ns/task measured vs
~150 ns for fp16) and fp16 also halves the dominant l1_emb DMA traffic.
Accumulation (PSUM) and the whole epilogue stay fp32; measured end-to-end
L2 relative error ~3e-4.
"""

import numpy as np

import concourse.bacc as bacc
import concourse.mybir as mybir
import concourse.tile as tile
from concourse.bass_utils import run_bass_kernel_spmd

NUM_TASKS = 1000
N_FEATURES = 256
HIDDEN = 128
BATCH = 4096
N_CORES = 8
TPC = NUM_TASKS // N_CORES  # tasks per core = 125
GRP = 5                     # tasks per w1 DMA

INV_SQRT2 = float(1.0 / np.sqrt(2.0))

# Module-level knobs for the test harness (the grader just calls kernel()).
MM_DTYPE = "float16"  # "float16" (fast path) or "float32" (exact fallback)
TRACE = False
TMPDIR = None  # optional fixed artifact dir for profiling runs
SIM_CORES = None  # e.g. [0]: run CoreSim for those cores instead of hardware
SIM_EXECUTOR_CLS = None  # optional InstructionExecutor subclass for CoreSim
LAST_RESULTS = None

_PROGRAM_CACHE = {}


def _build_program(W, mm_dtype):
    """Emit the SPMD Tile program for slot width W (W must divide 512)."""
    assert 512 % W == 0
    GB = (512 // W // GRP) * GRP  # tasks per PSUM block (GB*W <= 512, GRP | GB)
    assert GB >= GRP
    # block sizes; the last blocks are kept small so the epilogue chain after
    # the final matmul (the only part that can't hide under DMA) is short
    sizes = [GB] * (TPC // GB)
    if TPC % GB:
        sizes.append(TPC % GB)
    if sizes[-1] >= 4 * GRP:
        last = sizes.pop()
        sizes += [last - 3 * GRP, GRP, GRP, GRP]
    NSLOT = TPC * W
    f32 = mybir.dt.float32
    mdt = getattr(mybir.dt, mm_dtype)

    nc = bacc.Bacc("TRN2", target_bir_lowering=False, debug=False)

    xT_d = nc.dram_tensor("xT", [2, 128, NSLOT], mdt, kind="ExternalInput").ap()
    # w1 slab, host-packed partition-major: [group, partition, task, chunk, h]
    w1_d = nc.dram_tensor(
        "w1s", [TPC // GRP, 128, GRP, 2, 128], mdt, kind="ExternalInput"
    ).ap()
    b1_d = nc.dram_tensor("b1Ts", [128, TPC], f32, kind="ExternalInput").ap()
    w2_d = nc.dram_tensor("w2T", [128, TPC], f32, kind="ExternalInput").ap()
    b2_d = nc.dram_tensor("b2r", [1, TPC], f32, kind="ExternalInput").ap()
    out_d = nc.dram_tensor("out", [1, NSLOT], f32, kind="ExternalOutput").ap()

    Erf = mybir.ActivationFunctionType.Erf
    add = mybir.AluOpType.add
    mult = mybir.AluOpType.mult

    with tile.TileContext(nc) as tc:
        with (
            tc.tile_pool(name="const", bufs=1) as constp,
            tc.tile_pool(name="w1pool", bufs=8) as w1p,
            tc.tile_pool(name="work", bufs=2) as workp,
            tc.tile_pool(name="hpsum", bufs=3, space="PSUM") as hpsp,
            tc.tile_pool(name="opsum", bufs=2, space="PSUM") as opsp,
        ):
            # x columns, transposed, as two K-chunks of [128, NSLOT].
            # Non-w1 traffic goes through SWDGE (gpsimd) so the sync HWDGE
            # ring carries nothing but the dominant w1 stream (HWDGE DMAs
            # execute FIFO per issuing engine).  The sync ring's first DMA
            # can't start until the Tile preamble barrier (~7 us), so the
            # first two w1 groups also go via SWDGE, which starts at ~2.5 us
            # — the PE can begin while the sync ring is still warming up.
            xc0 = constp.tile([128, NSLOT], mdt)
            nc.gpsimd.dma_start(out=xc0, in_=xT_d[0])
            xc1 = constp.tile([128, NSLOT], mdt)
            nc.gpsimd.dma_start(out=xc1, in_=xT_d[1])

            cones = constp.tile([128, 1], f32)
            nc.vector.memset(cones, INV_SQRT2)

            out_sb = constp.tile([1, NSLOT], f32)

            b1T = w2T = b2r = None
            gdma = 0
            for b, gbt in enumerate(sizes):
                g0 = sum(sizes[:b])
                cols = gbt * W
                base = g0 * W
                csl = slice(base, base + cols)

                ps = hpsp.tile([128, cols], mybir.dt.float32, tag="hps")
                for gi in range(gbt // GRP):
                    grp = (g0 + gi * GRP) // GRP
                    w1t = w1p.tile([128, GRP, 2, 128], mdt, tag="w1t")
                    eng = nc.gpsimd if gdma < 6 else nc.sync
                    eng.dma_start(out=w1t, in_=w1_d[grp])
                    gdma += 1
                    if gdma == 2:
                        # consts ride SWDGE after the bootstrap w1 groups;
                        # they're not needed until the first epilogue
                        b1T = constp.tile([128, TPC], f32)
                        nc.gpsimd.dma_start(out=b1T, in_=b1_d)
                        w2T = constp.tile([128, TPC], f32)
                        nc.gpsimd.dma_start(out=w2T, in_=w2_d)
                        b2r = constp.tile([1, TPC], f32)
                        nc.gpsimd.dma_start(out=b2r, in_=b2_d)
                    for j in range(GRP):
                        jj = gi * GRP + j
                        sl = slice(jj * W, (jj + 1) * W)
                        xsl = slice(base + jj * W, base + (jj + 1) * W)
                        nc.tensor.matmul(
                            ps[:, sl], lhsT=w1t[:, j, 0], rhs=xc0[:, xsl],
                            start=True, stop=False,
                        )
                        nc.tensor.matmul(
                            ps[:, sl], lhsT=w1t[:, j, 1], rhs=xc1[:, xsl],
                            start=False, stop=True,
                        )

                # hs = h / sqrt(2)  (b1Ts is host-scaled by 1/sqrt(2))
                hs = workp.tile([128, cols], f32, tag="hs")
                b1v = b1T[:, g0:g0 + gbt].unsqueeze(2).broadcast_to([128, gbt, W])
                nc.vector.scalar_tensor_tensor(
                    hs.rearrange("p (g w) -> p g w", w=W),
                    ps.rearrange("p (g w) -> p g w", w=W),
                    INV_SQRT2, b1v, op0=mult, op1=add,
                )
                # e = erf(hs)
                esb = workp.tile([128, cols], f32, tag="esb")
                nc.scalar.activation(esb, hs, Erf)
                # tt = (e + 1) * hs = sqrt(2) * gelu(h)   (in-place into esb)
                nc.vector.scalar_tensor_tensor(
                    esb, esb, 1.0, hs, op0=add, op1=mult,
                )
                # prod = tt * w2 (column-broadcast view), in-place into hs
                w2v = w2T[:, g0:g0 + gbt].unsqueeze(2).broadcast_to([128, gbt, W])
                nc.vector.tensor_mul(
                    hs.rearrange("p (g w) -> p g w", w=W),
                    esb.rearrange("p (g w) -> p g w", w=W),
                    w2v,
                )
                # reduce over hidden: [1, cols] = (1/sqrt(2)).T @ prod
                ops = opsp.tile([1, cols], mybir.dt.float32, tag="ops")
                nc.tensor.matmul(ops, lhsT=cones, rhs=hs, start=True, stop=True)
                # + b2 (column-broadcast), into the output staging tile
                b2v = b2r[:, g0:g0 + gbt].unsqueeze(2).broadcast_to([1, gbt, W])
                nc.vector.tensor_add(
                    out_sb[:, csl].rearrange("p (g w) -> p g w", w=W),
                    ops.rearrange("p (g w) -> p g w", w=W),
                    b2v,
                )
                nc.gpsimd.dma_start(out=out_d[:, csl], in_=out_sb[:, csl])

    nc.compile()
    return nc


def _get_program(W, mm_dtype):
    key = (W, mm_dtype)
    if key not in _PROGRAM_CACHE:
        _PROGRAM_CACHE[key] = _build_program(W, mm_dtype)
    return _PROGRAM_CACHE[key]


def kernel(x, task_ids, l1_emb, l1_bias, l2_emb, l2_bias):
    global LAST_RESULTS
    x = np.ascontiguousarray(np.asarray(x, dtype=np.float32))
    tid = np.asarray(task_ids).astype(np.int64)
    l1_emb = np.ascontiguousarray(np.asarray(l1_emb, dtype=np.float32))
    l1_bias = np.ascontiguousarray(np.asarray(l1_bias, dtype=np.float32))
    l2_emb = np.ascontiguousarray(np.asarray(l2_emb, dtype=np.float32))
    l2_bias = np.ascontiguousarray(np.asarray(l2_bias, dtype=np.float32))

    B = x.shape[0]
    assert x.shape == (BATCH, N_FEATURES) and tid.shape == (BATCH,)

    mdt = np.float16 if MM_DTYPE == "float16" else np.float32

    counts = np.bincount(tid, minlength=NUM_TASKS)
    kmax = int(counts.max())
    if kmax <= 16:
        W = 16
    elif kmax <= 32:
        W = 32
    else:
        raise NotImplementedError(f"group size {kmax} > 32 unsupported")
    NSLOT = TPC * W

    # slot routing: sample j -> (core, slot) with slot = (task % TPC)*W + occ
    order = np.argsort(tid, kind="stable")
    sorted_tid = tid[order]
    starts = np.flatnonzero(np.r_[True, np.diff(sorted_tid) != 0])
    run_len = np.diff(np.r_[starts, B])
    run_pos = np.arange(B) - np.repeat(starts, run_len)
    occ = np.empty(B, dtype=np.int64)
    occ[order] = run_pos
    core = tid // TPC
    slot = (tid % TPC) * W + occ

    # scatter x into per-core transposed, padded slot grids
    xT = np.zeros((N_CORES, N_FEATURES, NSLOT), dtype=mdt)
    xT[core, :, slot] = x.astype(mdt)

    inv = np.float32(INV_SQRT2)
    in_maps = []
    for c in range(N_CORES):
        t0 = c * TPC
        sl = slice(t0, t0 + TPC)
        # [TPC, 2, 128, 128] -> [TPC/GRP, 128, GRP, 2, 128] partition-major
        w1s = (
            l1_emb[sl]
            .reshape(TPC // GRP, GRP, 2, 128, 128)
            .transpose(0, 3, 1, 2, 4)
            .astype(mdt)
        )
        in_maps.append({
            "xT": np.ascontiguousarray(xT[c].reshape(2, 128, NSLOT)),
            "w1s": np.ascontiguousarray(w1s),
            "b1Ts": np.ascontiguousarray(l1_bias[sl].T) * inv,
            "w2T": np.ascontiguousarray(l2_emb[sl].T),
            "b2r": np.ascontiguousarray(l2_bias[sl].reshape(1, TPC)),
        })

    nc = _get_program(W, MM_DTYPE)
    if SIM_CORES is not None:
        from concourse.bass_interp import CoreSim

        sim_results = []
        for c in range(N_CORES):
            if c in SIM_CORES:
                kw = {}
                if SIM_EXECUTOR_CLS is not None:
                    kw["executor_cls"] = SIM_EXECUTOR_CLS
                sim = CoreSim(nc, publish_trace=False, **kw)
                for k, v in in_maps[c].items():
                    sim.tensor(k)[:] = v
                sim.simulate()
                sim_results.append({"out": np.array(sim.tensor("out"))})
            else:
                sim_results.append({"out": np.zeros((1, NSLOT), np.float32)})
        outs = np.stack([r["out"].reshape(NSLOT) for r in sim_results])
        logits = outs[core, slot]
        return logits[:, None].astype(np.float32)

    res = run_bass_kernel_spmd(
        nc, in_maps, core_ids=list(range(N_CORES)), trace=TRACE, tmpdir=TMPDIR,
    )
    LAST_RESULTS = res

    outs = np.stack([r["out"].reshape(NSLOT) for r in res.results])
    logits = outs[core, slot]
    return logits[:, None].astype(np.float32)
